# revision 31
# baseline (speedup 1.0000x reference)
"""AttentionMILPooling Trainium2 kernel (single-X-copy design).

Math (matches the jax reference):
    scores  = tanh(X @ W1 + b1) @ W2 + b2          # [T, 1]
    weights = softmax(scores, axis=0)              # global over all T
    out[b]  = sum_{i in bag b} weights[i] * X[i]   # [64, 512]

Identities:
  * b2 cancels in the softmax -> dropped.
  * |scores| <= sum|W2| ~ 13, exp fits fp32/bf16 range -> no max-subtract.
  * out[b] = U[b] / Z with U[b] = sum_{i in b} exp(s_i) X_i and
    Z = sum_i exp(s_i); each core computes U for its 8 whole bags plus
    per-group partial sums of exp(s); the host sums Z and divides once.

Design: ONLY the transposed X is streamed (X^T bf16, features on
partitions) -- 16.8MB/core, half the DMA of the previous two-copy
design.  All compute that needs rows-on-partitions is eliminated:

  PE : H^T[m,i] = sum_c W1c^T @ X^T_c  (8 matmuls/group, 512-col moving)
  ACT: th = tanh(H^T + b1) -> bf16     (2 instrs/group, per-m bias)
  PE : s_bcast = w2rep^T @ th          (2 matmuls/group).  The stationary
       w2rep[p, j] = W2[m*128+p] is column-replicated, so every output
       partition j receives the same row s[i] -- the scores arrive
       already broadcast across all 128 partitions, no transpose and no
       separate broadcast pass.
  ACT: wsave[:, g] = exp(s_bcast) -> bf16 (replicated), with
       accum_out=z[g] giving the group's softmax-denominator partial.
  DVE: scalar_tensor_tensor(out=scratch, (X^T_c * 1) * wsave,
       accum_out=u[p, c, blk]) -- fused multiply + free-axis reduction,
       one instruction per (block, chunk) (the Pool engine rejects the
       TensorScalarPtr opcode, so DVE does all four chunks; STT runs at
       1 elem/partition/cycle -- no 2x uop exists for it).  The last
       block runs per-group so the drain tail stays short.  Partials are
       summed per-bag on the host (bag = 1 block = 4 groups).

Per 512-row group the engine budget is DVE 2.26us (bound), PE 2.13us,
ACT 2.0us, DMA 1.6us -- ~73us projected vs 112us for the two-copy
baseline.
"""

import numpy as np
import ml_dtypes

N_CORES = 8
F = 512  # feature dim
HID = 256  # hidden dim
P = 128  # partitions
BLK = 2048  # rows per DMA block (= bag size on the device path)
GR = 512  # rows per processing group
FC = F // P  # 4 feature chunks
MC = HID // P  # 2 hidden chunks

_COMPILED_CACHE = {}


def _group_stt_blocks(n_blocks):
    """Blocks whose weighted sums run per-group on DVE (early pipeline
    start for the first blocks -- DVE only goes backlogged around block
    4 -- and a short drain for the last); the rest run as one fused
    instruction per (block, chunk), which has 4x less overhead."""
    return {0, 1, 2, 3, n_blocks - 1} & set(range(n_blocks))


def _build_program(n_tiles):
    """Build the SPMD bass program.

    n_tiles: number of 128-row tiles per core; rows = n_tiles*128 must be
    divisible by BLK.
    """
    import concourse.bacc as bacc
    import concourse.mybir as mybir
    from concourse.tile import TileContext

    f32 = mybir.dt.float32
    bf16 = mybir.dt.bfloat16
    rows = n_tiles * P
    n_groups = rows // GR
    n_blocks = rows // BLK
    GPB = BLK // GR  # groups per block
    LOOKAHEAD = 2

    nc = bacc.Bacc(
        "TRN2", target_bir_lowering=False, debug=False, num_devices=N_CORES
    )

    xt = nc.declare_dram_parameter("xt", [F, rows], bf16, isOutput=False)
    w1 = nc.declare_dram_parameter("w1", [P, FC, MC, P], bf16, isOutput=False)
    b1 = nc.declare_dram_parameter("b1", [P, MC], f32, isOutput=False)
    w2r = nc.declare_dram_parameter("w2r", [P, MC, P], bf16, isOutput=False)
    u_out = nc.declare_dram_parameter("u", [P, FC, n_groups], f32, isOutput=True)
    z_out = nc.declare_dram_parameter("z", [1, n_groups], f32, isOutput=True)

    with TileContext(nc) as tc:
        with (
            tc.tile_pool(name="const", bufs=1) as const_pool,
            tc.tile_pool(name="xt", bufs=4) as xt_pool,
            tc.tile_pool(name="th", bufs=3) as th_pool,
            tc.tile_pool(name="yv", bufs=2) as yv_pool,
            tc.tile_pool(name="yg", bufs=2) as yg_pool,
            tc.tile_pool(name="hp", bufs=2, space="PSUM") as hp_pool,
            tc.tile_pool(name="sp", bufs=2, space="PSUM") as sp_pool,
        ):
            xt_hist = {}
            # ---- constants (host pre-chunked) ----
            # Issue order is the startup critical path: the first H matmul
            # needs w1b (m=0 half) and block 0's first quarter, so those
            # go first, split so several DMA queues fill in parallel
            # (each dma_start costs ~0.7us of SP sequencer issue time,
            # serialized).
            # w1b[p, c, m, j] = W1[c*128+p, m*128+j]
            w1b = const_pool.tile([P, FC, MC, P], bf16)
            nc.sync.dma_start(out=w1b[:, :, 0, :], in_=w1[:, :, 0, :])

            # exp(scores), partition-replicated; read back only via U/z.
            wsave = const_pool.tile([P, n_groups * GR], bf16)
            # weighted-sum partials, indexed by group.  Group-granular
            # blocks (first two: early DVE start; last: short drain)
            # fill all their group slots; block-granular blocks write one
            # whole-bag sum into their first group's slot and leave the
            # rest unwritten (the host never reads those).
            u_sb = const_pool.tile([P, FC, n_groups], f32)
            # per-group softmax-denominator partials (replicated rows).
            z_sb = const_pool.tile([P, n_groups], f32)

            def emit_load(bb, quarters=None):
                xtt = xt_pool.tile([P, FC, BLK], bf16, name="xt", tag="xt")
                xt_hist[bb] = xtt
                if quarters is not None:
                    # quarter-DMAs so the first H matmuls start early
                    for h in quarters:
                        nc.sync.dma_start(
                            out=xtt[:, :, h * GR : (h + 1) * GR],
                            in_=xt[
                                :, bb * BLK + h * GR : bb * BLK + (h + 1) * GR
                            ].rearrange("(c p) i -> p c i", p=P),
                        )
                else:
                    nc.sync.dma_start(
                        out=xtt,
                        in_=xt[:, bb * BLK : (bb + 1) * BLK].rearrange(
                            "(c p) i -> p c i", p=P
                        ),
                    )
                return xtt

            # block 0's first quarter lands per-chunk right after w1b-m0:
            # H(0) accumulates c = 0..3 in order, so its first matmul
            # starts once chunk 0 (128KB) is in, while c1-c3 stream.
            xtt0 = xt_pool.tile([P, FC, BLK], bf16, name="xt", tag="xt")
            xt_hist[0] = xtt0
            for c in range(FC):
                nc.sync.dma_start(
                    out=xtt0[:, c, 0:GR],
                    in_=xt[c * P : (c + 1) * P, 0:GR],
                )

            nc.sync.dma_start(out=w1b[:, :, 1, :], in_=w1[:, :, 1, :])
            # b1s[p, m] = b1[m*128+p] (tanh(0) needs it right after H(0))
            b1s = const_pool.tile([P, MC], f32)
            nc.sync.dma_start(out=b1s, in_=b1[:, :])
            # w2b[p, m, j] = W2[m*128+p] for every j (column-replicated)
            w2b = const_pool.tile([P, MC, P], bf16)
            nc.sync.dma_start(out=w2b, in_=w2r[:, :, :])

            for h in range(1, GPB):
                nc.sync.dma_start(
                    out=xtt0[:, :, h * GR : (h + 1) * GR],
                    in_=xt[:, h * GR : (h + 1) * GR].rearrange(
                        "(c p) i -> p c i", p=P
                    ),
                )
            for bb in range(1, min(LOOKAHEAD + 1, n_blocks)):
                emit_load(bb)

            # PE p-state warmup: the tensor engine ramps 0.65 -> 2.4 GHz
            # over ~3us of continuous execution.  A dependency-free chain
            # of dummy matmuls on uninitialized SBUF (result never read)
            # burns the ramp while the first DMAs are still in flight, so
            # the first real H matmuls run at full clock.
            warm = const_pool.tile([P, GR], bf16)
            nc.gpsimd.memset(warm, 0)
            wp = sp_pool.tile([P, GR], f32, name="wp", tag="sp")
            for k in range(10):
                nc.tensor.matmul(
                    wp,
                    warm[:, 0:P],
                    warm,
                    start=(k == 0),
                    stop=(k == 9),
                )
            # also trigger the scalar engine's lazy ACT_TABLE_LOAD
            # (~1.3us) now instead of blocking the first real tanh.
            warm_a = const_pool.tile([P, 4], bf16)
            nc.scalar.activation(
                warm_a, warm[:, 0:4], mybir.ActivationFunctionType.Tanh
            )

            th_hist = {}

            def emit_s(gg):
                # s_bcast[j, i] = sum_m W2[m] th[m, i] for every j: the
                # column-replicated stationary makes all 128 output
                # partitions identical, i.e. scores pre-broadcast.
                th_g = th_hist.pop(gg)
                sp = sp_pool.tile([P, GR], f32, name="sp", tag="sp")
                for m in range(MC):
                    nc.tensor.matmul(
                        sp,
                        w2b[:, m, :],
                        th_g[:, m, :],
                        start=(m == 0),
                        stop=(m == MC - 1),
                    )
                nc.scalar.activation(
                    wsave[:, gg * GR : (gg + 1) * GR],
                    sp,
                    mybir.ActivationFunctionType.Exp,
                    accum_out=z_sb[:, gg : gg + 1],
                )

            group_stt = _group_stt_blocks(n_blocks)

            def emit_u_block(bb):
                # fused multiply + free-axis accumulate over a whole
                # block (= bag): u[p, c, 4bb] = sum_i X^T[c*128+p, i]*w[i]
                xtb = xt_hist.pop(bb)
                wsl = wsave[:, bb * BLK : (bb + 1) * BLK]
                for c in range(FC):
                    y = yv_pool.tile([P, BLK], bf16, name="y", tag="y")
                    nc.vector.scalar_tensor_tensor(
                        out=y,
                        in0=xtb[:, c, :],
                        scalar=1.0,
                        in1=wsl,
                        op0=mybir.AluOpType.mult,
                        op1=mybir.AluOpType.mult,
                        accum_out=u_sb[:, c, bb * GPB : bb * GPB + 1],
                    )

            def emit_u_group(gg):
                # per-group variant: runs as soon as the group's exp is
                # out, instead of waiting for the whole block's wsave.
                bb, h = divmod(gg, GPB)
                xtb = xt_hist[bb]
                wsl = wsave[:, gg * GR : (gg + 1) * GR]
                for c in range(FC):
                    y = yg_pool.tile([P, GR], bf16, name="yg", tag="yg")
                    nc.vector.scalar_tensor_tensor(
                        out=y,
                        in0=xtb[:, c, h * GR : (h + 1) * GR],
                        scalar=1.0,
                        in1=wsl,
                        op0=mybir.AluOpType.mult,
                        op1=mybir.AluOpType.mult,
                        accum_out=u_sb[:, c, gg : gg + 1],
                    )
                if h == GPB - 1:
                    del xt_hist[bb]

            # ---- main loop over 512-row groups (software-pipelined) ----
            for g in range(n_groups):
                bb, h = divmod(g, GPB)
                if h == 0 and (bb + LOOKAHEAD) < n_blocks and (
                    bb + LOOKAHEAD
                ) not in xt_hist:
                    emit_load(bb + LOOKAHEAD)
                xtb = xt_hist[bb]

                hp = hp_pool.tile([P, MC, GR], f32, name="hp", tag="hp")
                for m in range(MC):
                    for c in range(FC):
                        nc.tensor.matmul(
                            hp[:, m, :],
                            w1b[:, c, m, :],
                            xtb[:, c, h * GR : (h + 1) * GR],
                            start=(c == 0),
                            stop=(c == FC - 1),
                        )
                th = th_pool.tile([P, MC, GR], bf16, name="th", tag="th")
                th_hist[g] = th
                for m in range(MC):
                    nc.scalar.activation(
                        th[:, m, :],
                        hp[:, m, :],
                        mybir.ActivationFunctionType.Tanh,
                        bias=b1s[:, m : m + 1],
                    )

                if g == 0:
                    # early: PE waits ~0.7us on tanh(0) here, but the
                    # first weighted-sum lands on DVE two groups sooner.
                    emit_s(0)
                    emit_u_group(0)
                if g >= 2:
                    emit_s(g - 1)
                    pb, ph = divmod(g - 1, GPB)
                    if pb in group_stt:
                        emit_u_group(g - 1)
                    elif ph == GPB - 1:
                        emit_u_block(pb)
                        if pb == n_blocks - 2:
                            # all u slots before the last block are now
                            # in flight; stream them out under the tail.
                            nc.sync.dma_start(
                                out=u_out[:, :, : pb * GPB + 1],
                                in_=u_sb[:, :, : pb * GPB + 1],
                            )

            emit_s(n_groups - 1)
            # all u slots except the final group's are now in flight;
            # stream them (and all but the last z) under the last STTs so
            # the end-of-kernel DMA is just one slot + one z column.
            last = (n_blocks - 2) * GPB + 1
            nc.sync.dma_start(
                out=u_out[:, :, last : n_groups - 1],
                in_=u_sb[:, :, last : n_groups - 1],
            )
            nc.sync.dma_start(
                out=z_out[:, : n_groups - 1], in_=z_sb[0:1, : n_groups - 1]
            )
            emit_u_group(n_groups - 1)

            nc.sync.dma_start(
                out=u_out[:, :, n_groups - 1 :], in_=u_sb[:, :, n_groups - 1 :]
            )
            nc.sync.dma_start(
                out=z_out[:, n_groups - 1 :], in_=z_sb[0:1, n_groups - 1 :]
            )

    nc.compile()
    return nc


def _run_device(X, W1, b1, W2, bag_rows, trace=False, trace_kwargs=None):
    from concourse.bass_utils import run_bass_kernel_spmd

    rows_per_core = X.shape[0] // N_CORES
    n_tiles = rows_per_core // P
    n_groups = rows_per_core // GR
    gpb = bag_rows // GR  # groups per bag
    n_bags_core = rows_per_core // bag_rows

    key = rows_per_core
    if key in _COMPILED_CACHE:
        nc = _COMPILED_CACHE[key]
    else:
        nc = _build_program(n_tiles)
        _COMPILED_CACHE[key] = nc

    w1b = np.ascontiguousarray(
        np.asarray(W1, np.float32).reshape(FC, P, MC, P).transpose(1, 0, 2, 3)
    ).astype(ml_dtypes.bfloat16)
    # w2r[p, m, j] = W2[m*128+p] replicated along j
    w2r = np.ascontiguousarray(
        np.broadcast_to(
            np.asarray(W2, np.float32).reshape(MC, P, 1).transpose(1, 0, 2),
            (P, MC, P),
        )
    ).astype(ml_dtypes.bfloat16)
    b1s = np.ascontiguousarray(
        np.asarray(b1, np.float32).reshape(MC, P).T, np.float32
    )

    in_maps = []
    for c in range(N_CORES):
        xc = np.asarray(
            X[c * rows_per_core : (c + 1) * rows_per_core], np.float32
        )
        xt_c = np.ascontiguousarray(xc.T).astype(ml_dtypes.bfloat16)
        in_maps.append({"xt": xt_c, "w1": w1b, "b1": b1s, "w2r": w2r})
    kw = dict(trace_kwargs or {})
    res = run_bass_kernel_spmd(
        nc, in_maps, list(range(N_CORES)), trace=trace, **kw
    )

    group_stt = _group_stt_blocks(n_bags_core)
    U = np.zeros((N_CORES * n_bags_core, F), np.float32)
    Z = np.float64(0.0)
    for c in range(N_CORES):
        u = np.asarray(res.results[c]["u"], np.float32)  # [P, FC, n_groups]
        # group-granular bags: sum their gpb group slots; block-granular
        # bags: the whole-bag sum sits in their first group's slot.
        ub = np.stack(
            [
                u[:, :, b * gpb : (b + 1) * gpb].sum(axis=2)
                if b in group_stt
                else u[:, :, b * gpb]
                for b in range(n_bags_core)
            ],
            axis=2,
        )
        # U[b, cc*128+p] = ub[p, cc, b]
        U[c * n_bags_core : (c + 1) * n_bags_core] = (
            ub.transpose(2, 1, 0).reshape(n_bags_core, F)
        )
        Z += np.asarray(res.results[c]["z"], np.float64).sum()
    return U, Z, res


def _kernel_numpy(instance_features, bag_sizes, W1, b1, W2, b2):
    """Exact-math fallback for bag layouts the device program doesn't cover."""
    X = np.asarray(instance_features, np.float32)
    s = np.tanh(X @ W1 + b1) @ W2.reshape(-1, 1) + np.asarray(b2).reshape(1, -1)
    s = s - s.max()
    w = np.exp(s)
    w = w / w.sum()
    offsets = np.cumsum(np.asarray(bag_sizes, np.int64))
    seg = np.searchsorted(offsets, np.arange(X.shape[0]), side="right")
    out = np.zeros((len(bag_sizes), X.shape[1]), np.float32)
    np.add.at(out, seg[seg < len(bag_sizes)], (X * w)[seg < len(bag_sizes)])
    return out


def kernel(**inputs):
    X = np.asarray(inputs["instance_features"], np.float32)
    bag_sizes = np.asarray(inputs["bag_sizes"], np.int64)
    W1 = np.asarray(inputs["W1"], np.float32)
    b1 = np.asarray(inputs["b1"], np.float32)
    W2 = np.asarray(inputs["W2"], np.float32)
    b2 = np.asarray(inputs["b2"], np.float32)

    T, Fdim = X.shape
    B = bag_sizes.shape[0]
    bag = int(bag_sizes[0]) if B else 0
    # Device path constraints: equal whole bags per core, BLK-row DMA
    # blocks, bag == BLK (per-group U partials summed per-bag on host).
    aligned = (
        Fdim == F
        and B > 0
        and np.all(bag_sizes == bag)
        and bag == BLK
        and bag * B == T
        and T % N_CORES == 0
        and (T // N_CORES) % BLK == 0
    )
    if not aligned:
        return _kernel_numpy(X, bag_sizes, W1, b1, W2, b2)

    U, Z, _ = _run_device(X, W1, b1, W2, bag)
    return (U / np.float32(Z)).astype(np.float32)


# revision 33
# speedup vs baseline: 1.0297x; 1.0297x over previous
"""AttentionMILPooling Trainium2 kernel (single-X-copy design).

Math (matches the jax reference):
    scores  = tanh(X @ W1 + b1) @ W2 + b2          # [T, 1]
    weights = softmax(scores, axis=0)              # global over all T
    out[b]  = sum_{i in bag b} weights[i] * X[i]   # [64, 512]

Identities:
  * b2 cancels in the softmax -> dropped.
  * |scores| <= sum|W2| ~ 13, exp fits fp32/bf16 range -> no max-subtract.
  * out[b] = U[b] / Z with U[b] = sum_{i in b} exp(s_i) X_i and
    Z = sum_i exp(s_i); each core computes U for its 8 whole bags plus
    per-group partial sums of exp(s); the host sums Z and divides once.

Design: ONLY the transposed X is streamed (X^T bf16, features on
partitions) -- 16.8MB/core, half the DMA of the previous two-copy
design.  All compute that needs rows-on-partitions is eliminated:

  PE : H^T[m,i] = sum_c W1c^T @ X^T_c  (8 matmuls/group, 512-col moving)
  ACT: th = tanh(H^T + b1) -> bf16     (2 instrs/group, per-m bias)
  PE : s_bcast = w2rep^T @ th          (2 matmuls/group).  The stationary
       w2rep[p, j] = W2[m*128+p] is column-replicated, so every output
       partition j receives the same row s[i] -- the scores arrive
       already broadcast across all 128 partitions, no transpose and no
       separate broadcast pass.
  ACT: wsave[:, g] = exp(s_bcast) -> bf16 (replicated), with
       accum_out=z[g] giving the group's softmax-denominator partial.
  DVE: scalar_tensor_tensor(out=scratch, (X^T_c * 1) * wsave,
       accum_out=u[p, c, blk]) -- fused multiply + free-axis reduction,
       one instruction per (block, chunk) (the Pool engine rejects the
       TensorScalarPtr opcode, so DVE does all four chunks; STT runs at
       1 elem/partition/cycle -- no 2x uop exists for it).  The last
       block runs per-group so the drain tail stays short.  Partials are
       summed per-bag on the host (bag = 1 block = 4 groups).

Per 512-row group the engine budget is DVE 2.26us (bound), PE 2.13us,
ACT 2.0us, DMA 1.6us -- ~73us projected vs 112us for the two-copy
baseline.
"""

import numpy as np
import ml_dtypes

N_CORES = 8
F = 512  # feature dim
HID = 256  # hidden dim
P = 128  # partitions
BLK = 2048  # rows per DMA block (= bag size on the device path)
GR = 512  # rows per processing group
FC = F // P  # 4 feature chunks
MC = HID // P  # 2 hidden chunks

_COMPILED_CACHE = {}


def _group_stt_blocks(n_blocks):
    """Blocks whose weighted sums run per-group on DVE (early pipeline
    start for the first blocks -- DVE only goes backlogged around block
    4 -- and a short drain for the last); the rest run as one fused
    instruction per (block, chunk), which has 4x less overhead."""
    return {0, 1, 2, 3, 4, n_blocks - 1} & set(range(n_blocks))


def _build_program(n_tiles):
    """Build the SPMD bass program.

    n_tiles: number of 128-row tiles per core; rows = n_tiles*128 must be
    divisible by BLK.
    """
    import concourse.bacc as bacc
    import concourse.mybir as mybir
    from concourse.tile import TileContext

    f32 = mybir.dt.float32
    bf16 = mybir.dt.bfloat16
    rows = n_tiles * P
    n_groups = rows // GR
    n_blocks = rows // BLK
    GPB = BLK // GR  # groups per block
    LOOKAHEAD = 2

    nc = bacc.Bacc(
        "TRN2", target_bir_lowering=False, debug=False, num_devices=N_CORES
    )

    xt = nc.declare_dram_parameter("xt", [F, rows], bf16, isOutput=False)
    w1 = nc.declare_dram_parameter("w1", [P, FC, MC, P], bf16, isOutput=False)
    b1 = nc.declare_dram_parameter("b1", [P, MC], f32, isOutput=False)
    w2r = nc.declare_dram_parameter("w2r", [P, MC, P], bf16, isOutput=False)
    u_out = nc.declare_dram_parameter("u", [P, FC, n_groups], f32, isOutput=True)
    z_out = nc.declare_dram_parameter("z", [1, n_groups], f32, isOutput=True)

    with TileContext(nc) as tc:
        with (
            tc.tile_pool(name="const", bufs=1) as const_pool,
            tc.tile_pool(name="xt", bufs=4) as xt_pool,
            tc.tile_pool(name="th", bufs=3) as th_pool,
            tc.tile_pool(name="yv", bufs=2) as yv_pool,
            tc.tile_pool(name="yg", bufs=2) as yg_pool,
            tc.tile_pool(name="hp", bufs=2, space="PSUM") as hp_pool,
            tc.tile_pool(name="sp", bufs=2, space="PSUM") as sp_pool,
        ):
            xt_hist = {}
            # ---- constants (host pre-chunked) ----
            # Issue order is the startup critical path: the first H matmul
            # needs w1b (m=0 half) and block 0's first quarter, so those
            # go first, split so several DMA queues fill in parallel
            # (each dma_start costs ~0.7us of SP sequencer issue time,
            # serialized).
            # w1b[p, c, m, j] = W1[c*128+p, m*128+j]
            w1b = const_pool.tile([P, FC, MC, P], bf16)
            nc.sync.dma_start(out=w1b[:, :, 0, :], in_=w1[:, :, 0, :])

            # exp(scores), partition-replicated; read back only via U/z.
            wsave = const_pool.tile([P, n_groups * GR], bf16)
            # weighted-sum partials, indexed by group.  Group-granular
            # blocks (first two: early DVE start; last: short drain)
            # fill all their group slots; block-granular blocks write one
            # whole-bag sum into their first group's slot and leave the
            # rest unwritten (the host never reads those).
            u_sb = const_pool.tile([P, FC, n_groups], f32)
            # per-group softmax-denominator partials (replicated rows).
            z_sb = const_pool.tile([P, n_groups], f32)

            def emit_load(bb, quarters=None):
                xtt = xt_pool.tile([P, FC, BLK], bf16, name="xt", tag="xt")
                xt_hist[bb] = xtt
                if quarters is not None:
                    # quarter-DMAs so the first H matmuls start early
                    for h in quarters:
                        nc.sync.dma_start(
                            out=xtt[:, :, h * GR : (h + 1) * GR],
                            in_=xt[
                                :, bb * BLK + h * GR : bb * BLK + (h + 1) * GR
                            ].rearrange("(c p) i -> p c i", p=P),
                        )
                else:
                    nc.sync.dma_start(
                        out=xtt,
                        in_=xt[:, bb * BLK : (bb + 1) * BLK].rearrange(
                            "(c p) i -> p c i", p=P
                        ),
                    )
                return xtt

            # block 0's first quarter right after w1b-m0, so H(0) can
            # start while the remaining startup DMAs are still issuing.
            xtt0 = xt_pool.tile([P, FC, BLK], bf16, name="xt", tag="xt")
            xt_hist[0] = xtt0
            nc.sync.dma_start(
                out=xtt0[:, :, 0:GR],
                in_=xt[:, 0:GR].rearrange("(c p) i -> p c i", p=P),
            )

            nc.sync.dma_start(out=w1b[:, :, 1, :], in_=w1[:, :, 1, :])
            # b1s[p, m] = b1[m*128+p] (tanh(0) needs it right after H(0))
            b1s = const_pool.tile([P, MC], f32)
            nc.sync.dma_start(out=b1s, in_=b1[:, :])
            # w2b[p, m, j] = W2[m*128+p] for every j (column-replicated)
            w2b = const_pool.tile([P, MC, P], bf16)
            nc.sync.dma_start(out=w2b, in_=w2r[:, :, :])

            for h in range(1, GPB):
                nc.sync.dma_start(
                    out=xtt0[:, :, h * GR : (h + 1) * GR],
                    in_=xt[:, h * GR : (h + 1) * GR].rearrange(
                        "(c p) i -> p c i", p=P
                    ),
                )
            for bb in range(1, min(LOOKAHEAD + 1, n_blocks)):
                emit_load(bb)

            # PE p-state warmup: the tensor engine ramps 0.65 -> 2.4 GHz
            # over ~3us of continuous execution.  A dependency-free chain
            # of dummy matmuls on uninitialized SBUF (result never read)
            # burns the ramp while the first DMAs are still in flight, so
            # the first real H matmuls run at full clock.
            warm = const_pool.tile([P, GR], bf16)
            nc.gpsimd.memset(warm, 0)
            wp = sp_pool.tile([P, GR], f32, name="wp", tag="sp")
            for k in range(10):
                nc.tensor.matmul(
                    wp,
                    warm[:, 0:P],
                    warm,
                    start=(k == 0),
                    stop=(k == 9),
                )
            # also trigger the scalar engine's lazy ACT_TABLE_LOAD
            # (~1.3us) now instead of blocking the first real tanh.
            warm_a = const_pool.tile([P, 4], bf16)
            nc.scalar.activation(
                warm_a, warm[:, 0:4], mybir.ActivationFunctionType.Tanh
            )

            th_hist = {}

            def emit_s(gg):
                # s_bcast[j, i] = sum_m W2[m] th[m, i] for every j: the
                # column-replicated stationary makes all 128 output
                # partitions identical, i.e. scores pre-broadcast.
                th_g = th_hist.pop(gg)
                sp = sp_pool.tile([P, GR], f32, name="sp", tag="sp")
                for m in range(MC):
                    nc.tensor.matmul(
                        sp,
                        w2b[:, m, :],
                        th_g[:, m, :],
                        start=(m == 0),
                        stop=(m == MC - 1),
                    )
                nc.scalar.activation(
                    wsave[:, gg * GR : (gg + 1) * GR],
                    sp,
                    mybir.ActivationFunctionType.Exp,
                    accum_out=z_sb[:, gg : gg + 1],
                )

            group_stt = _group_stt_blocks(n_blocks)

            def emit_u_block(bb):
                # fused multiply + free-axis accumulate over a whole
                # block (= bag): u[p, c, 4bb] = sum_i X^T[c*128+p, i]*w[i]
                xtb = xt_hist.pop(bb)
                wsl = wsave[:, bb * BLK : (bb + 1) * BLK]
                for c in range(FC):
                    y = yv_pool.tile([P, BLK], bf16, name="y", tag="y")
                    nc.vector.scalar_tensor_tensor(
                        out=y,
                        in0=xtb[:, c, :],
                        scalar=1.0,
                        in1=wsl,
                        op0=mybir.AluOpType.mult,
                        op1=mybir.AluOpType.mult,
                        accum_out=u_sb[:, c, bb * GPB : bb * GPB + 1],
                    )

            def emit_u_group(gg):
                # per-group variant: runs as soon as the group's exp is
                # out, instead of waiting for the whole block's wsave.
                bb, h = divmod(gg, GPB)
                xtb = xt_hist[bb]
                wsl = wsave[:, gg * GR : (gg + 1) * GR]
                for c in range(FC):
                    y = yg_pool.tile([P, GR], bf16, name="yg", tag="yg")
                    nc.vector.scalar_tensor_tensor(
                        out=y,
                        in0=xtb[:, c, h * GR : (h + 1) * GR],
                        scalar=1.0,
                        in1=wsl,
                        op0=mybir.AluOpType.mult,
                        op1=mybir.AluOpType.mult,
                        accum_out=u_sb[:, c, gg : gg + 1],
                    )
                if h == GPB - 1:
                    del xt_hist[bb]

            # ---- main loop over 512-row groups (software-pipelined) ----
            for g in range(n_groups):
                bb, h = divmod(g, GPB)
                if h == 0 and (bb + LOOKAHEAD) < n_blocks and (
                    bb + LOOKAHEAD
                ) not in xt_hist:
                    emit_load(bb + LOOKAHEAD)
                xtb = xt_hist[bb]

                hp = hp_pool.tile([P, MC, GR], f32, name="hp", tag="hp")
                for m in range(MC):
                    for c in range(FC):
                        nc.tensor.matmul(
                            hp[:, m, :],
                            w1b[:, c, m, :],
                            xtb[:, c, h * GR : (h + 1) * GR],
                            start=(c == 0),
                            stop=(c == FC - 1),
                        )
                th = th_pool.tile([P, MC, GR], bf16, name="th", tag="th")
                th_hist[g] = th
                for m in range(MC):
                    nc.scalar.activation(
                        th[:, m, :],
                        hp[:, m, :],
                        mybir.ActivationFunctionType.Tanh,
                        bias=b1s[:, m : m + 1],
                    )

                if g == 0:
                    # early: PE waits ~0.7us on tanh(0) here, but the
                    # first weighted-sum lands on DVE two groups sooner.
                    emit_s(0)
                    emit_u_group(0)
                if g >= 2:
                    emit_s(g - 1)
                    pb, ph = divmod(g - 1, GPB)
                    if pb in group_stt:
                        emit_u_group(g - 1)
                    elif ph == GPB - 1:
                        emit_u_block(pb)
                        if pb == n_blocks - 2:
                            # all u slots before the last block are now
                            # in flight; stream them out under the tail.
                            nc.sync.dma_start(
                                out=u_out[:, :, : pb * GPB + 1],
                                in_=u_sb[:, :, : pb * GPB + 1],
                            )

            emit_s(n_groups - 1)
            # all u slots except the final group's are now in flight;
            # stream them (and all but the last z) under the last STTs so
            # the end-of-kernel DMA is just one slot + one z column.
            last = (n_blocks - 2) * GPB + 1
            nc.sync.dma_start(
                out=u_out[:, :, last : n_groups - 1],
                in_=u_sb[:, :, last : n_groups - 1],
            )
            nc.sync.dma_start(
                out=z_out[:, : n_groups - 1], in_=z_sb[0:1, : n_groups - 1]
            )
            emit_u_group(n_groups - 1)

            nc.sync.dma_start(
                out=u_out[:, :, n_groups - 1 :], in_=u_sb[:, :, n_groups - 1 :]
            )
            nc.sync.dma_start(
                out=z_out[:, n_groups - 1 :], in_=z_sb[0:1, n_groups - 1 :]
            )

    nc.compile()
    return nc


def _run_device(X, W1, b1, W2, bag_rows, trace=False, trace_kwargs=None):
    from concourse.bass_utils import run_bass_kernel_spmd

    rows_per_core = X.shape[0] // N_CORES
    n_tiles = rows_per_core // P
    n_groups = rows_per_core // GR
    gpb = bag_rows // GR  # groups per bag
    n_bags_core = rows_per_core // bag_rows

    key = rows_per_core
    if key in _COMPILED_CACHE:
        nc = _COMPILED_CACHE[key]
    else:
        nc = _build_program(n_tiles)
        _COMPILED_CACHE[key] = nc

    w1b = np.ascontiguousarray(
        np.asarray(W1, np.float32).reshape(FC, P, MC, P).transpose(1, 0, 2, 3)
    ).astype(ml_dtypes.bfloat16)
    # w2r[p, m, j] = W2[m*128+p] replicated along j
    w2r = np.ascontiguousarray(
        np.broadcast_to(
            np.asarray(W2, np.float32).reshape(MC, P, 1).transpose(1, 0, 2),
            (P, MC, P),
        )
    ).astype(ml_dtypes.bfloat16)
    b1s = np.ascontiguousarray(
        np.asarray(b1, np.float32).reshape(MC, P).T, np.float32
    )

    in_maps = []
    for c in range(N_CORES):
        xc = np.asarray(
            X[c * rows_per_core : (c + 1) * rows_per_core], np.float32
        )
        xt_c = np.ascontiguousarray(xc.T).astype(ml_dtypes.bfloat16)
        in_maps.append({"xt": xt_c, "w1": w1b, "b1": b1s, "w2r": w2r})
    kw = dict(trace_kwargs or {})
    res = run_bass_kernel_spmd(
        nc, in_maps, list(range(N_CORES)), trace=trace, **kw
    )

    group_stt = _group_stt_blocks(n_bags_core)
    U = np.zeros((N_CORES * n_bags_core, F), np.float32)
    Z = np.float64(0.0)
    for c in range(N_CORES):
        u = np.asarray(res.results[c]["u"], np.float32)  # [P, FC, n_groups]
        # group-granular bags: sum their gpb group slots; block-granular
        # bags: the whole-bag sum sits in their first group's slot.
        ub = np.stack(
            [
                u[:, :, b * gpb : (b + 1) * gpb].sum(axis=2)
                if b in group_stt
                else u[:, :, b * gpb]
                for b in range(n_bags_core)
            ],
            axis=2,
        )
        # U[b, cc*128+p] = ub[p, cc, b]
        U[c * n_bags_core : (c + 1) * n_bags_core] = (
            ub.transpose(2, 1, 0).reshape(n_bags_core, F)
        )
        Z += np.asarray(res.results[c]["z"], np.float64).sum()
    return U, Z, res


def _kernel_numpy(instance_features, bag_sizes, W1, b1, W2, b2):
    """Exact-math fallback for bag layouts the device program doesn't cover."""
    X = np.asarray(instance_features, np.float32)
    s = np.tanh(X @ W1 + b1) @ W2.reshape(-1, 1) + np.asarray(b2).reshape(1, -1)
    s = s - s.max()
    w = np.exp(s)
    w = w / w.sum()
    offsets = np.cumsum(np.asarray(bag_sizes, np.int64))
    seg = np.searchsorted(offsets, np.arange(X.shape[0]), side="right")
    out = np.zeros((len(bag_sizes), X.shape[1]), np.float32)
    np.add.at(out, seg[seg < len(bag_sizes)], (X * w)[seg < len(bag_sizes)])
    return out


def kernel(**inputs):
    X = np.asarray(inputs["instance_features"], np.float32)
    bag_sizes = np.asarray(inputs["bag_sizes"], np.int64)
    W1 = np.asarray(inputs["W1"], np.float32)
    b1 = np.asarray(inputs["b1"], np.float32)
    W2 = np.asarray(inputs["W2"], np.float32)
    b2 = np.asarray(inputs["b2"], np.float32)

    T, Fdim = X.shape
    B = bag_sizes.shape[0]
    bag = int(bag_sizes[0]) if B else 0
    # Device path constraints: equal whole bags per core, BLK-row DMA
    # blocks, bag == BLK (per-group U partials summed per-bag on host).
    aligned = (
        Fdim == F
        and B > 0
        and np.all(bag_sizes == bag)
        and bag == BLK
        and bag * B == T
        and T % N_CORES == 0
        and (T // N_CORES) % BLK == 0
    )
    if not aligned:
        return _kernel_numpy(X, bag_sizes, W1, b1, W2, b2)

    U, Z, _ = _run_device(X, W1, b1, W2, bag)
    return (U / np.float32(Z)).astype(np.float32)
